# revision 5
# baseline (speedup 1.0000x reference)
"""BitNetLinear Trainium2 kernel (8 NeuronCores, SPMD data-parallel).

y = round(clip(x, +-127*s)/s)*s @ (ternary(W))^T + ternary(b)
with s = exp2(floor(log2(max|x|/127 + eps))) a power of two (global over x).

Sharding: batch dim (8) -> one batch of [4096, 1024] per core.
Host prep: x shard transposed to [in, rows] (PE contracts over partitions);
weight/bias ternary-quantized on host (reference does this in __init__);
ternary weight shipped as bf16 [in, out].

Device: phase 1 streams x computing local absmax -> partition_all_reduce ->
512B AllReduce(max) across the 8 cores; scale = exponent-masked (exact
power of two); x quantized to integer-valued bf16 (round-half-even via
+-1.5*2^23 trick); bf16 matmul with fp32 PSUM accumulation is exact integer
arithmetic (|x_int| <= 127, w in {-1,0,1}, |acc| < 2^24); result scaled by
s*gamma and bias added.
"""

import numpy as np
import ml_dtypes
from contextlib import ExitStack

import concourse.bass as bass
import concourse.mybir as mybir
import concourse.tile as tile
from concourse import bacc, bass_isa, bass_utils

F32 = mybir.dt.float32
BF16 = mybir.dt.bfloat16
I32 = mybir.dt.int32

N_CORES = 8
P = 128
IN_F = 1024
OUT_F = 1024
KC = IN_F // P          # 8 contraction chunks
RSUB = 256              # rows loaded/quantized per chunk
ROUND_C = 12582912.0    # 1.5 * 2**23: float32 round-half-even trick
EPS = 1e-8
QMAX = 127.0


def build_program(rows: int = 4096, num_cores: int = N_CORES) -> bacc.Bacc:
    assert rows % RSUB == 0
    nc = bacc.Bacc(
        "TRN2",
        target_bir_lowering=False,
        debug=False,
        enable_asserts=False,
        num_devices=num_cores,
    )
    xt = nc.dram_tensor("xt", (IN_F, rows), F32, kind="ExternalInput").ap()
    wq = nc.dram_tensor("wq", (IN_F, OUT_F), BF16, kind="ExternalInput").ap()
    bq = nc.dram_tensor("bq", (1, OUT_F), F32, kind="ExternalInput").ap()
    gq = nc.dram_tensor("gq", (1, 1), F32, kind="ExternalInput").ap()
    y = nc.dram_tensor("y", (rows, OUT_F), F32, kind="ExternalOutput").ap()
    # Collectives cannot target I/O tensors; bounce through internal DRAM.
    cc_in = nc.dram_tensor("cc_in", (P, 1), F32).ap()
    cc_out = nc.dram_tensor("cc_out", (P, 1), F32).ap()

    with tile.TileContext(nc, num_cores=num_cores) as tc, ExitStack() as ctx:
        consts = ctx.enter_context(tc.tile_pool(name="consts", bufs=1))

        # --- constants: ternary weight [p, kc, o], bias row, gamma scalar ---
        w_sb = consts.tile([P, KC, OUT_F], BF16)
        nc.sync.dma_start(out=w_sb, in_=wq.rearrange("(c p) o -> p c o", p=P))
        bias_sb = consts.tile([P, OUT_F], F32)
        nc.sync.dma_start(out=bias_sb, in_=bq.to_broadcast((P, OUT_F)))
        gamma_sb = consts.tile([P, 1], F32)
        nc.sync.dma_start(out=gamma_sb, in_=gq.to_broadcast((P, 1)))

        # --- phase 1: local absmax of the x shard ---
        xt_chunks = xt.rearrange("(c p) r -> c p r", p=P)  # [KC, P, rows]
        partials = consts.tile([P, KC], F32)
        with tc.tile_pool(name="xmax", bufs=3) as xpool:
            for j in range(KC):
                xsb = xpool.tile([P, rows], F32)
                nc.sync.dma_start(out=xsb, in_=xt_chunks[j])
                nc.vector.tensor_reduce(
                    out=partials[:, j : j + 1],
                    in_=xsb,
                    axis=mybir.AxisListType.X,
                    op=mybir.AluOpType.max,
                    apply_absolute_value=True,
                )
        lmax = consts.tile([P, 1], F32)
        nc.vector.tensor_reduce(
            out=lmax, in_=partials, axis=mybir.AxisListType.X, op=mybir.AluOpType.max
        )
        gmax_l = consts.tile([P, 1], F32)
        nc.gpsimd.partition_all_reduce(
            gmax_l, lmax, channels=P, reduce_op=bass_isa.ReduceOp.max
        )

        # --- global max across the 8 cores ---
        nc.sync.dma_start(out=cc_in, in_=gmax_l)
        nc.gpsimd.collective_compute(
            "AllReduce",
            mybir.AluOpType.max,
            replica_groups=[list(range(num_cores))],
            ins=[cc_in.opt()],
            outs=[cc_out.opt()],
        )
        gmax = consts.tile([P, 1], F32)
        nc.sync.dma_start(out=gmax, in_=cc_out)

        # --- scale: s = exp2(floor(log2(m/127 + eps))) via exponent masking ---
        v_t = consts.tile([P, 1], F32)
        nc.vector.tensor_scalar(
            out=v_t,
            in0=gmax,
            scalar1=float(np.float32(1.0 / 127.0)),
            scalar2=float(np.float32(EPS)),
            op0=mybir.AluOpType.mult,
            op1=mybir.AluOpType.add,
        )
        mask_t = consts.tile([P, 1], I32)
        nc.vector.memset(mask_t, -8388608)  # 0xFF800000: sign+exponent mask
        expc_t = consts.tile([P, 1], I32)
        nc.vector.memset(expc_t, 0x7F000000)  # bits of (254<<23)
        s_t = consts.tile([P, 1], F32)
        nc.vector.tensor_tensor(
            out=s_t.bitcast(I32),
            in0=v_t.bitcast(I32),
            in1=mask_t,
            op=mybir.AluOpType.bitwise_and,
        )
        # 1/s for a power of two: exponent bits of (254<<23) - s_bits
        inv_t = consts.tile([P, 1], F32)
        nc.vector.tensor_tensor(
            out=inv_t.bitcast(I32),
            in0=expc_t,
            in1=s_t.bitcast(I32),
            op=mybir.AluOpType.subtract,
        )
        c_t = consts.tile([P, 1], F32)  # s * gamma_w
        nc.vector.tensor_mul(out=c_t, in0=s_t, in1=gamma_sb)
        negc_t = consts.tile([P, 1], F32)
        nc.vector.memset(negc_t, -ROUND_C)

        # --- phase 2: quantize + matmul + scale/bias + store ---
        # per 256-row chunk: [P(in), KC, RSUB] tiles of x^T
        xt_cols = xt.rearrange("(c p) (t r) -> t p c r", p=P, r=RSUB)
        y_rows = y.rearrange("(t p) o -> t p o", p=P)
        nhalf = OUT_F // 512
        with (
            tc.tile_pool(name="xq", bufs=3) as xq_pool,
            tc.tile_pool(name="tq", bufs=2) as tq_pool,
            tc.tile_pool(name="ub", bufs=2) as ub_pool,
            tc.tile_pool(name="xi", bufs=3) as xi_pool,
            tc.tile_pool(name="yo", bufs=4) as yo_pool,
            tc.tile_pool(name="ps", bufs=4, space="PSUM") as ps_pool,
        ):
            for t in range(rows // RSUB):
                xc = xq_pool.tile([P, KC, RSUB], F32)
                nc.sync.dma_start(out=xc, in_=xt_cols[t])
                # t = x/s + C  (mult is exact: s a power of two)
                tq = tq_pool.tile([P, KC, RSUB], F32)
                nc.vector.tensor_scalar(
                    out=tq,
                    in0=xc,
                    scalar1=inv_t,
                    scalar2=ROUND_C,
                    op0=mybir.AluOpType.mult,
                    op1=mybir.AluOpType.add,
                )
                # u = t - C  -> integer-valued, cast to bf16 (exact, |u|<256)
                ub = ub_pool.tile([P, KC, RSUB], BF16)
                nc.scalar.activation(
                    out=ub,
                    in_=tq,
                    func=mybir.ActivationFunctionType.Identity,
                    bias=negc_t,
                    scale=1.0,
                )
                # x_int = clip(u, -127, 127)
                xi = xi_pool.tile([P, KC, RSUB], BF16)
                nc.vector.tensor_scalar(
                    out=xi,
                    in0=ub,
                    scalar1=-127.0,
                    scalar2=127.0,
                    op0=mybir.AluOpType.max,
                    op1=mybir.AluOpType.min,
                )
                for h in range(RSUB // P):
                    ps = ps_pool.tile([P, OUT_F], F32)
                    for k in range(KC):
                        for n in range(nhalf):
                            nc.tensor.matmul(
                                ps[:, n * 512 : (n + 1) * 512],
                                lhsT=xi[:, k, h * P : (h + 1) * P],
                                rhs=w_sb[:, k, n * 512 : (n + 1) * 512],
                                start=(k == 0),
                                stop=(k == KC - 1),
                            )
                    yo = yo_pool.tile([P, OUT_F], F32)
                    nc.scalar.activation(
                        out=yo,
                        in_=ps,
                        func=mybir.ActivationFunctionType.Copy,
                        bias=0.0,
                        scale=c_t,
                    )
                    nc.vector.tensor_add(out=yo, in0=yo, in1=bias_sb)
                    nc.sync.dma_start(
                        out=y_rows[t * (RSUB // P) + h], in_=yo
                    )

    nc.compile()
    return nc


def quantize_params(weight: np.ndarray, bias: np.ndarray):
    """Ternary-quantize weight/bias exactly as the reference (f64 math whose
    f32 rounding matches jax-f32; verified margins are orders of magnitude
    above f32 accumulation differences)."""
    w64 = weight.astype(np.float64)
    g_w = np.float32(np.abs(w64).mean())
    wi = np.clip(np.round(w64 / (np.float64(g_w) + EPS)), -1.0, 1.0)
    b64 = bias.astype(np.float64)
    g_b = np.float32(np.abs(b64).mean())
    bi = np.clip(np.round(b64 / (np.float64(g_b) + EPS)), -1.0, 1.0)
    bq = (bi * np.float64(g_b)).astype(np.float32)  # exact: {-g_b, 0, g_b}
    return wi, g_w, bq


_PROGRAM_CACHE: dict[int, bacc.Bacc] = {}


def _get_program(rows: int) -> bacc.Bacc:
    if rows not in _PROGRAM_CACHE:
        _PROGRAM_CACHE[rows] = build_program(rows)
    return _PROGRAM_CACHE[rows]


def prepare_in_maps(x: np.ndarray, weight: np.ndarray, bias: np.ndarray):
    x = np.asarray(x, dtype=np.float32)
    weight = np.asarray(weight, dtype=np.float32)
    bias = np.asarray(bias, dtype=np.float32)
    batch, rows, in_f = x.shape
    assert batch == N_CORES and in_f == IN_F and weight.shape == (OUT_F, IN_F)

    wi, g_w, bq = quantize_params(weight, bias)
    wq_t = np.ascontiguousarray(wi.T).astype(ml_dtypes.bfloat16)  # [in, out]
    bq_row = np.ascontiguousarray(bq.reshape(1, OUT_F))
    gq = np.array([[g_w]], dtype=np.float32)

    in_maps = []
    for c in range(N_CORES):
        in_maps.append(
            {
                "xt": np.ascontiguousarray(x[c].T),
                "wq": wq_t,
                "bq": bq_row,
                "gq": gq,
            }
        )
    return in_maps, rows


def kernel(x: np.ndarray, weight: np.ndarray, bias: np.ndarray) -> np.ndarray:
    in_maps, rows = prepare_in_maps(x, weight, bias)
    nc = _get_program(rows)
    res = bass_utils.run_bass_kernel_spmd(nc, in_maps, core_ids=list(range(N_CORES)))
    return np.stack([res.results[c]["y"] for c in range(N_CORES)], axis=0)


# revision 9
# speedup vs baseline: 1.0142x; 1.0142x over previous
"""BitNetLinear Trainium2 kernel (8 NeuronCores, SPMD data-parallel).

y = round(clip(x, +-127*s)/s)*s @ (ternary(W))^T + ternary(b)
with s = exp2(floor(log2(max|x|/127 + eps))) a power of two (global over x).

Sharding: batch dim (8) -> one batch of [4096, 1024] per core.
Host prep: x shard transposed to [in, rows] (PE contracts over partitions);
weight/bias ternary-quantized on host (reference does this in __init__);
ternary weight shipped as bf16 [in, out].

Device: phase 1 streams x computing local absmax -> partition_all_reduce ->
512B AllReduce(max) across the 8 cores; scale = exponent-masked (exact
power of two); x quantized to integer-valued bf16 (round-half-even via
+-1.5*2^23 trick); bf16 matmul with fp32 PSUM accumulation is exact integer
arithmetic (|x_int| <= 127, w in {-1,0,1}, |acc| < 2^24); result scaled by
s*gamma and bias added.
"""

import numpy as np
import ml_dtypes
from contextlib import ExitStack

import concourse.bass as bass
import concourse.mybir as mybir
import concourse.tile as tile
from concourse import bacc, bass_isa, bass_utils

F32 = mybir.dt.float32
BF16 = mybir.dt.bfloat16
I32 = mybir.dt.int32

N_CORES = 8
P = 128
IN_F = 1024
OUT_F = 1024
KC = IN_F // P          # 8 contraction chunks
RSUB = 256              # rows loaded/quantized per chunk
ROUND_C = 12582912.0    # 1.5 * 2**23: float32 round-half-even trick
EPS = 1e-8
QMAX = 127.0


def build_program(rows: int = 4096, num_cores: int = N_CORES) -> bacc.Bacc:
    assert rows % RSUB == 0
    nc = bacc.Bacc(
        "TRN2",
        target_bir_lowering=False,
        debug=False,
        enable_asserts=False,
        num_devices=num_cores,
    )
    xt = nc.dram_tensor("xt", (IN_F, rows), F32, kind="ExternalInput").ap()
    wq = nc.dram_tensor("wq", (IN_F, OUT_F), BF16, kind="ExternalInput").ap()
    bq = nc.dram_tensor("bq", (1, OUT_F), F32, kind="ExternalInput").ap()
    gq = nc.dram_tensor("gq", (1, 1), F32, kind="ExternalInput").ap()
    y = nc.dram_tensor("y", (rows, OUT_F), F32, kind="ExternalOutput").ap()
    # Collectives cannot target I/O tensors; bounce through internal DRAM.
    cc_in = nc.dram_tensor("cc_in", (P, 1), F32).ap()
    cc_out = nc.dram_tensor("cc_out", (P, 1), F32).ap()

    with tile.TileContext(nc, num_cores=num_cores) as tc, ExitStack() as ctx:
        consts = ctx.enter_context(tc.tile_pool(name="consts", bufs=1))

        # --- phase 1 first: local absmax of the x shard (keeps the serial
        # prefix to the AllReduce free of const-load DMA traffic) ---
        half = rows // 2
        xt_chunks = xt.rearrange("(c p) (h r) -> c h p r", p=P, h=2)
        partials = consts.tile([P, 2 * KC], F32)
        with tc.tile_pool(name="xmax", bufs=4) as xpool:
            for j in range(2 * KC):
                xsb = xpool.tile([P, half], F32)
                nc.sync.dma_start(out=xsb, in_=xt_chunks[j // 2, j % 2])
                nc.vector.tensor_reduce(
                    out=partials[:, j : j + 1],
                    in_=xsb,
                    axis=mybir.AxisListType.X,
                    op=mybir.AluOpType.max,
                    apply_absolute_value=True,
                )

        # --- constants: ternary weight [p, kc, o], bias row, gamma scalar ---
        w_sb = consts.tile([P, KC, OUT_F], BF16)
        nc.sync.dma_start(out=w_sb, in_=wq.rearrange("(c p) o -> p c o", p=P))
        bias_sb = consts.tile([P, OUT_F], F32)
        nc.sync.dma_start(out=bias_sb, in_=bq.to_broadcast((P, OUT_F)))
        gamma_sb = consts.tile([P, 1], F32)
        nc.sync.dma_start(out=gamma_sb, in_=gq.to_broadcast((P, 1)))
        lmax = consts.tile([P, 1], F32)
        nc.vector.tensor_reduce(
            out=lmax,
            in_=partials,
            axis=mybir.AxisListType.X,
            op=mybir.AluOpType.max,
            apply_absolute_value=True,
        )
        gmax_l = consts.tile([P, 1], F32)
        nc.gpsimd.partition_all_reduce(
            gmax_l, lmax, channels=P, reduce_op=bass_isa.ReduceOp.max
        )

        # --- global max across the 8 cores ---
        nc.sync.dma_start(out=cc_in, in_=gmax_l)
        nc.gpsimd.collective_compute(
            "AllReduce",
            mybir.AluOpType.max,
            replica_groups=[list(range(num_cores))],
            ins=[cc_in.opt()],
            outs=[cc_out.opt()],
        )
        gmax = consts.tile([P, 1], F32)
        nc.sync.dma_start(out=gmax, in_=cc_out)

        # --- PE warmup: junk matmuls gated on the AllReduce result. They
        # fill the post-collective bubble while the scale chain + first
        # quantize run, flipping HAM to full clock before the real matmuls.
        warm_rhs = consts.tile([P, 512], BF16)
        nc.vector.memset(warm_rhs, 0.0)
        nc.vector.tensor_copy(out=warm_rhs[:, 0:1], in_=gmax)
        with tc.tile_pool(name="warm_ps", bufs=1, space="PSUM") as warm_pool:
            warm_ps = warm_pool.tile([P, 512], F32)
            for _ in range(16):
                nc.tensor.matmul(
                    warm_ps,
                    lhsT=w_sb[:, 0, 0:P],
                    rhs=warm_rhs,
                    start=True,
                    stop=True,
                )

        # --- scale: s = exp2(floor(log2(m/127 + eps))) via exponent masking ---
        v_t = consts.tile([P, 1], F32)
        nc.vector.tensor_scalar(
            out=v_t,
            in0=gmax,
            scalar1=float(np.float32(1.0 / 127.0)),
            scalar2=float(np.float32(EPS)),
            op0=mybir.AluOpType.mult,
            op1=mybir.AluOpType.add,
        )
        mask_t = consts.tile([P, 1], I32)
        nc.vector.memset(mask_t, -8388608)  # 0xFF800000: sign+exponent mask
        expc_t = consts.tile([P, 1], I32)
        nc.vector.memset(expc_t, 0x7F000000)  # bits of (254<<23)
        s_t = consts.tile([P, 1], F32)
        nc.vector.tensor_tensor(
            out=s_t.bitcast(I32),
            in0=v_t.bitcast(I32),
            in1=mask_t,
            op=mybir.AluOpType.bitwise_and,
        )
        # 1/s for a power of two: exponent bits of (254<<23) - s_bits
        inv_t = consts.tile([P, 1], F32)
        nc.vector.tensor_tensor(
            out=inv_t.bitcast(I32),
            in0=expc_t,
            in1=s_t.bitcast(I32),
            op=mybir.AluOpType.subtract,
        )
        c_t = consts.tile([P, 1], F32)  # s * gamma_w
        nc.vector.tensor_mul(out=c_t, in0=s_t, in1=gamma_sb)
        negc_t = consts.tile([P, 1], F32)
        nc.vector.memset(negc_t, -ROUND_C)

        # --- phase 2: quantize + matmul + scale/bias + store ---
        # per 256-row chunk: [P(in), KC, RSUB] tiles of x^T
        xt_cols = xt.rearrange("(c p) (t r) -> t p c r", p=P, r=RSUB)
        y_rows = y.rearrange("(t p) o -> t p o", p=P)
        nhalf = OUT_F // 512
        with (
            tc.tile_pool(name="xq", bufs=6) as xq_pool,
            tc.tile_pool(name="tq", bufs=3) as tq_pool,
            tc.tile_pool(name="ub", bufs=3) as ub_pool,
            tc.tile_pool(name="xi", bufs=4) as xi_pool,
            tc.tile_pool(name="yo", bufs=4) as yo_pool,
            tc.tile_pool(name="ps", bufs=4, space="PSUM") as ps_pool,
        ):
            for t in range(rows // RSUB):
                xc = xq_pool.tile([P, KC, RSUB], F32)
                nc.sync.dma_start(out=xc, in_=xt_cols[t])
                # t = x/s + C  (mult is exact: s a power of two)
                tq = tq_pool.tile([P, KC, RSUB], F32)
                nc.vector.tensor_scalar(
                    out=tq,
                    in0=xc,
                    scalar1=inv_t,
                    scalar2=ROUND_C,
                    op0=mybir.AluOpType.mult,
                    op1=mybir.AluOpType.add,
                )
                # u = t - C  -> integer-valued, cast to bf16 (exact, |u|<256)
                ub = ub_pool.tile([P, KC, RSUB], BF16)
                nc.scalar.activation(
                    out=ub,
                    in_=tq,
                    func=mybir.ActivationFunctionType.Identity,
                    bias=negc_t,
                    scale=1.0,
                )
                # x_int = clip(u, -127, 127)
                xi = xi_pool.tile([P, KC, RSUB], BF16)
                nc.vector.tensor_scalar(
                    out=xi,
                    in0=ub,
                    scalar1=-127.0,
                    scalar2=127.0,
                    op0=mybir.AluOpType.max,
                    op1=mybir.AluOpType.min,
                )
                for h in range(RSUB // P):
                    ps = ps_pool.tile([P, OUT_F], F32)
                    for k in range(KC):
                        for n in range(nhalf):
                            nc.tensor.matmul(
                                ps[:, n * 512 : (n + 1) * 512],
                                lhsT=xi[:, k, h * P : (h + 1) * P],
                                rhs=w_sb[:, k, n * 512 : (n + 1) * 512],
                                start=(k == 0),
                                stop=(k == KC - 1),
                            )
                    yo = yo_pool.tile([P, OUT_F], F32)
                    nc.scalar.activation(
                        out=yo,
                        in_=ps,
                        func=mybir.ActivationFunctionType.Copy,
                        bias=0.0,
                        scale=c_t,
                    )
                    nc.vector.tensor_add(out=yo, in0=yo, in1=bias_sb)
                    nc.sync.dma_start(
                        out=y_rows[t * (RSUB // P) + h], in_=yo
                    )

    nc.compile()
    return nc


def quantize_params(weight: np.ndarray, bias: np.ndarray):
    """Ternary-quantize weight/bias exactly as the reference (f64 math whose
    f32 rounding matches jax-f32; verified margins are orders of magnitude
    above f32 accumulation differences)."""
    w64 = weight.astype(np.float64)
    g_w = np.float32(np.abs(w64).mean())
    wi = np.clip(np.round(w64 / (np.float64(g_w) + EPS)), -1.0, 1.0)
    b64 = bias.astype(np.float64)
    g_b = np.float32(np.abs(b64).mean())
    bi = np.clip(np.round(b64 / (np.float64(g_b) + EPS)), -1.0, 1.0)
    bq = (bi * np.float64(g_b)).astype(np.float32)  # exact: {-g_b, 0, g_b}
    return wi, g_w, bq


_PROGRAM_CACHE: dict[int, bacc.Bacc] = {}


def _get_program(rows: int) -> bacc.Bacc:
    if rows not in _PROGRAM_CACHE:
        _PROGRAM_CACHE[rows] = build_program(rows)
    return _PROGRAM_CACHE[rows]


def prepare_in_maps(x: np.ndarray, weight: np.ndarray, bias: np.ndarray):
    x = np.asarray(x, dtype=np.float32)
    weight = np.asarray(weight, dtype=np.float32)
    bias = np.asarray(bias, dtype=np.float32)
    batch, rows, in_f = x.shape
    assert batch == N_CORES and in_f == IN_F and weight.shape == (OUT_F, IN_F)

    wi, g_w, bq = quantize_params(weight, bias)
    wq_t = np.ascontiguousarray(wi.T).astype(ml_dtypes.bfloat16)  # [in, out]
    bq_row = np.ascontiguousarray(bq.reshape(1, OUT_F))
    gq = np.array([[g_w]], dtype=np.float32)

    in_maps = []
    for c in range(N_CORES):
        in_maps.append(
            {
                "xt": np.ascontiguousarray(x[c].T),
                "wq": wq_t,
                "bq": bq_row,
                "gq": gq,
            }
        )
    return in_maps, rows


def kernel(x: np.ndarray, weight: np.ndarray, bias: np.ndarray) -> np.ndarray:
    in_maps, rows = prepare_in_maps(x, weight, bias)
    nc = _get_program(rows)
    res = bass_utils.run_bass_kernel_spmd(nc, in_maps, core_ids=list(range(N_CORES)))
    return np.stack([res.results[c]["y"] for c in range(N_CORES)], axis=0)
